# revision 6
# baseline (speedup 1.0000x reference)
"""Trainium2 Bass kernel for the vq_codebook bag-classification model.

Math (per reference):
  h1 = lrelu(x @ Wi.T + bi); h2 = lrelu(h1 @ Wh.T + bh); z = lrelu(h2 @ Wz.T + bz)
  c2p = ||z||^2 - 2 z@P.T + ||P||^2 ;  r = 1/(c2p + 0.5)
  c_logits = r @ Wclf.T ; segment-mean over bags ; CE loss.

Device strategy (8 cores, data-parallel over cells):
  - x (fp32, HBM) is loaded with an SWDGE cast-DMA (fp32 -> bf16 inline,
    natural [cells, dims] layout, fully contiguous HBM reads), then each
    [128, 128] block is transposed SBUF->SBUF via the DMA xbar so the
    encoder matmuls get dims-on-partitions bf16 tiles.
  - Encoder runs as bf16 matmuls producing transposed activations
    (feature-on-partition), leaky-relu + bias fused on the Scalar engine.
  - Distances via an augmented matmul: lhsT = [z; z^2; 1; 1] (66 x cells),
    rhs = [-2 P^T; ones; psq_hi; psq_lo] (66 x 64) -> c2p + 0.5 in PSUM.
  - r = reciprocal_approx_fast (DVE). Pooling = matmul with a one-hot
    bag matrix built from segment ids (is_equal against an iota row),
    accumulated in one PSUM bank across the whole kernel -> [64, 65]
    (64 proto-sums per bag + count column).
  - Host: sum the 8 per-core [64,65] partials, apply Wclf, divide by
    counts, log-softmax + NLL. (All-reduce equivalent, trivially small.)
"""

import sys

sys.path.insert(0, "/opt/trn_rl_repo")

import numpy as np
import ml_dtypes

from concourse import bass, bacc, tile, mybir
from concourse.bass_utils import run_bass_kernel_spmd

F32 = mybir.dt.float32
BF16 = mybir.dt.bfloat16

N_CORES = 8
N_CELLS = 200000
D_IN = 1000
H_DIM = 256
Z_DIM = 32
N_PROTO = 64
N_CLASSES = 8
N_BAGS = 64

CELLS_CORE = N_CELLS // N_CORES  # 25000
GROUP = 512  # cells per device group (4 subtiles of 128)

# K-chunking of the 1000-dim input: 7 chunks of 128 + one final chunk
# covering dims 872..999 (its first 24 weight rows are zeroed because dims
# 872..895 are already covered by chunk 6).
K_STARTS = [128 * k for k in range(7)] + [D_IN - 128]


def _group_offsets(cells):
    """Start offsets of 512-cell groups covering [0, cells); the last group
    is shifted back to stay in-bounds (duplicated cells are masked via seg
    ids on the host side)."""
    offs = list(range(0, cells - GROUP + 1, GROUP))
    if offs[-1] + GROUP < cells:
        offs.append(cells - GROUP)
    return offs


def build_program(cells=CELLS_CORE, num_devices=N_CORES, lrelu_native=True):
    offs = _group_offsets(cells)
    ng = len(offs)
    nc = bacc.Bacc(
        "TRN2", target_bir_lowering=False, debug=False, num_devices=num_devices
    )

    x = nc.declare_dram_parameter("x", [cells, D_IN], F32, isOutput=False)
    seg = nc.declare_dram_parameter("seg", [ng, 128, 4], F32, isOutput=False)
    wi = nc.declare_dram_parameter("wi", [8, 128, H_DIM], BF16, isOutput=False)
    wh = nc.declare_dram_parameter("wh", [2, 128, H_DIM], BF16, isOutput=False)
    wz = nc.declare_dram_parameter("wz", [2, 128, Z_DIM], BF16, isOutput=False)
    dw = nc.declare_dram_parameter("dw", [66, N_PROTO], BF16, isOutput=False)
    bi = nc.declare_dram_parameter("bi", [128, 2], F32, isOutput=False)
    bh = nc.declare_dram_parameter("bh", [128, 2], F32, isOutput=False)
    bz = nc.declare_dram_parameter("bz", [Z_DIM, 1], F32, isOutput=False)
    iota = nc.declare_dram_parameter("iota", [128, N_BAGS], F32, isOutput=False)
    out = nc.declare_dram_parameter("out", [N_BAGS, N_PROTO + 1], F32, isOutput=True)

    LRELU = mybir.ActivationFunctionType.Lrelu
    RELU = mybir.ActivationFunctionType.Relu

    with tile.TileContext(nc) as tc:
        with (
            tc.tile_pool(name="const", bufs=1) as cpool,
            tc.tile_pool(name="xt", bufs=3) as xt_pool,
            tc.tile_pool(name="act", bufs=3) as act_pool,
            tc.tile_pool(name="small", bufs=3) as small_pool,
            tc.tile_pool(name="ph1", bufs=1, space="PSUM") as ph1,
            tc.tile_pool(name="ph2", bufs=1, space="PSUM") as ph2,
            tc.tile_pool(name="pz", bufs=1, space="PSUM") as pz,
            tc.tile_pool(name="pd", bufs=1, space="PSUM") as pd,
            tc.tile_pool(name="pacc", bufs=1, space="PSUM") as pacc,
        ):
            # ---- constants / weights into SBUF (one-time) ----
            wi_sb = cpool.tile([128, 8, H_DIM], BF16)
            for k in range(8):
                nc.sync.dma_start(wi_sb[:, k, :], wi[k])
            wh_sb = cpool.tile([128, 2, H_DIM], BF16)
            for k in range(2):
                nc.sync.dma_start(wh_sb[:, k, :], wh[k])
            wz_sb = cpool.tile([128, 2, Z_DIM], BF16)
            for k in range(2):
                nc.sync.dma_start(wz_sb[:, k, :], wz[k])
            dw_sb = cpool.tile([66, N_PROTO], BF16)
            nc.sync.dma_start(dw_sb[:], dw[:])
            bi_sb = cpool.tile([128, 2], F32)
            nc.sync.dma_start(bi_sb[:], bi[:])
            bh_sb = cpool.tile([128, 2], F32)
            nc.sync.dma_start(bh_sb[:], bh[:])
            bz_sb = cpool.tile([Z_DIM, 1], F32)
            nc.sync.dma_start(bz_sb[:], bz[:])
            iota_sb = cpool.tile([128, N_BAGS], F32)
            nc.sync.dma_start(iota_sb[:], iota[:])

            pool_ps = pacc.tile([N_BAGS, N_PROTO + 1], F32)

            def lrelu(dst, src, bias_ap):
                if lrelu_native:
                    nc.scalar.activation(
                        dst, src, LRELU, bias=bias_ap, scale=1.0, alpha=0.01
                    )
                else:
                    # decomposed fallback: lrelu(t) = 0.01*t + 0.99*relu(t)
                    tmp = act_pool.tile(
                        [dst.shape[0], dst.shape[-1]], BF16, tag="lrelu_tmp"
                    )
                    nc.scalar.activation(tmp[:], src, RELU, bias=bias_ap, scale=1.0)
                    lin = act_pool.tile(
                        [dst.shape[0], dst.shape[-1]], BF16, tag="lrelu_lin"
                    )
                    nc.vector.tensor_scalar(
                        lin[:], src, bias_ap, 0.01, mybir.AluOpType.add,
                        mybir.AluOpType.mult,
                    )
                    # dst = 0.99*tmp + lin
                    nc.vector.scalar_tensor_tensor(
                        dst, tmp[:], 0.99, lin[:],
                        op0=mybir.AluOpType.mult, op1=mybir.AluOpType.add,
                    )

            for g, off in enumerate(offs):
                # --- cast-load x naturally, then xbar-transpose each block ---
                nat = []
                for cb in range(4):
                    n = xt_pool.tile([128, D_IN], BF16, tag=f"nat{cb}")
                    nc.gpsimd.dma_start(
                        n[:], x[off + 128 * cb : off + 128 * (cb + 1), :]
                    )
                    nat.append(n)
                xt = []
                for k in range(8):
                    t = xt_pool.tile([128, GROUP], BF16, tag=f"xt{k}")
                    s = K_STARTS[k]
                    for cb in range(4):
                        nc.sync.dma_start(
                            t[:, 128 * cb : 128 * (cb + 1)],
                            nat[cb][:, s : s + 128],
                            transpose=True,
                        )
                    xt.append(t)
                seg_t = small_pool.tile([128, 4], F32, tag="seg")
                nc.sync.dma_start(seg_t[:], seg[g])

                # --- layer 1: h1T[h] [128, 512] = lrelu(Wi @ xT + bi) ---
                h1sb = []
                for h in range(2):
                    ps = ph1.tile([128, GROUP], F32, tag=f"h1_{h}")
                    for k in range(8):
                        nc.tensor.matmul(
                            ps[:],
                            lhsT=wi_sb[:, k, 128 * h : 128 * h + 128],
                            rhs=xt[k][:],
                            start=(k == 0),
                            stop=(k == 7),
                        )
                    sbt = act_pool.tile([128, GROUP], BF16, tag=f"h1sb_{h}")
                    lrelu(sbt[:], ps[:], bi_sb[:, h : h + 1])
                    h1sb.append(sbt)

                # --- layer 2 ---
                h2sb = []
                for h in range(2):
                    ps = ph2.tile([128, GROUP], F32, tag=f"h2_{h}")
                    for k in range(2):
                        nc.tensor.matmul(
                            ps[:],
                            lhsT=wh_sb[:, k, 128 * h : 128 * h + 128],
                            rhs=h1sb[k][:],
                            start=(k == 0),
                            stop=(k == 1),
                        )
                    sbt = act_pool.tile([128, GROUP], BF16, tag=f"h2sb_{h}")
                    lrelu(sbt[:], ps[:], bh_sb[:, h : h + 1])
                    h2sb.append(sbt)

                # --- layer 3 -> dist_in rows 0:32 (z), 32:64 (z^2), 64:66 (1) ---
                zps = pz.tile([Z_DIM, GROUP], F32, tag="z")
                for k in range(2):
                    nc.tensor.matmul(
                        zps[:],
                        lhsT=wz_sb[:, k, :],
                        rhs=h2sb[k][:],
                        start=(k == 0),
                        stop=(k == 1),
                    )
                din = act_pool.tile([66, GROUP], BF16, tag="dist_in")
                lrelu(din[0:Z_DIM, :], zps[:], bz_sb[:])
                nc.vector.tensor_tensor(
                    din[Z_DIM : 2 * Z_DIM, :],
                    din[0:Z_DIM, :],
                    din[0:Z_DIM, :],
                    mybir.AluOpType.mult,
                )
                nc.gpsimd.memset(din[64:66, :], 1.0)

                # --- distances + 0.5 : c2p [128, 4, 64] per 128-cell subtile ---
                c2p = pd.tile([128, 4, N_PROTO], F32, tag="c2p")
                for j in range(4):
                    nc.tensor.matmul(
                        c2p[:, j, :],
                        lhsT=din[:, 128 * j : 128 * j + 128],
                        rhs=dw_sb[:],
                        start=True,
                        stop=True,
                    )

                # --- r = 1/(c2p+0.5), ones column, one-hot bags ---
                r_sb = small_pool.tile([128, 4, N_PROTO + 1], F32, tag="r")
                nc.vector.reciprocal_approx_fast(
                    out=r_sb[:, :, 0:N_PROTO], in_=c2p[:]
                )
                nc.gpsimd.memset(r_sb[:, :, N_PROTO : N_PROTO + 1], 1.0)
                oh = small_pool.tile([128, 4, N_BAGS], F32, tag="oh")
                for j in range(4):
                    nc.vector.tensor_scalar(
                        oh[:, j, :],
                        iota_sb[:],
                        seg_t[:, j : j + 1],
                        None,
                        mybir.AluOpType.is_equal,
                    )

                # --- pooled sums += onehot.T @ [r | 1] ---
                for j in range(4):
                    nc.tensor.matmul(
                        pool_ps[:],
                        lhsT=oh[:, j, :],
                        rhs=r_sb[:, j, :],
                        start=(g == 0 and j == 0),
                        stop=(g == ng - 1 and j == 3),
                    )

            out_sb = cpool.tile([N_BAGS, N_PROTO + 1], F32)
            nc.vector.tensor_copy(out_sb[:], pool_ps[:])
            nc.sync.dma_start(out[:], out_sb[:])

    nc.compile()
    return nc, offs, ng


def make_host_inputs(x, segment_ids, W_i, b_i, W_h, b_h, W_z, b_z, prototypes,
                     cells=CELLS_CORE, n_cores=N_CORES):
    """Build the per-core in_maps (and shared weight arrays)."""
    offs = _group_offsets(cells)
    ng = len(offs)

    xv = np.ascontiguousarray(np.asarray(x, dtype=np.float32))

    WiT = np.asarray(W_i, np.float32).T  # [1000, 256]
    wi = np.zeros((8, 128, H_DIM), np.float32)
    for k in range(7):
        wi[k] = WiT[128 * k : 128 * k + 128]
    wi[7] = WiT[D_IN - 128 : D_IN]
    n_overlap = 128 * 7 - (D_IN - 128)  # dims already covered by chunk 6
    wi[7][:n_overlap] = 0.0
    wi_bf = wi.astype(ml_dtypes.bfloat16)

    WhT = np.asarray(W_h, np.float32).T.reshape(2, 128, H_DIM)
    wh_bf = WhT.astype(ml_dtypes.bfloat16)
    WzT = np.asarray(W_z, np.float32).T.reshape(2, 128, Z_DIM)
    wz_bf = WzT.astype(ml_dtypes.bfloat16)

    P = np.asarray(prototypes, np.float32)
    dwf = np.zeros((66, N_PROTO), np.float32)
    dwf[0:32] = -2.0 * P.T
    dwf[32:64] = 1.0
    psq = (P * P).sum(1) + 0.5
    hi = psq.astype(ml_dtypes.bfloat16).astype(np.float32)
    dwf[64] = hi
    dwf[65] = psq - hi
    dw_bf = dwf.astype(ml_dtypes.bfloat16)

    bi_s = np.ascontiguousarray(np.asarray(b_i, np.float32).reshape(2, 128).T)
    bh_s = np.ascontiguousarray(np.asarray(b_h, np.float32).reshape(2, 128).T)
    bz_s = np.asarray(b_z, np.float32).reshape(Z_DIM, 1)
    iota = np.tile(np.arange(N_BAGS, dtype=np.float32), (128, 1))
    iota = np.ascontiguousarray(iota)

    seg_all = np.asarray(segment_ids).astype(np.float32)
    in_maps = []
    for c in range(n_cores):
        xc = xv[c * cells : (c + 1) * cells]
        sc = seg_all[c * cells : (c + 1) * cells]
        segf = np.empty((ng, GROUP), np.float32)
        for g, off in enumerate(offs):
            segf[g] = sc[off : off + GROUP]
        if ng >= 2:
            dup = offs[-2] + GROUP - offs[-1]  # cells of last group already done
            if dup > 0:
                segf[ng - 1, :dup] = -1.0  # matches no bag -> masked out
        seg_h = np.ascontiguousarray(segf.reshape(ng, 4, 128).transpose(0, 2, 1))
        in_maps.append(
            {
                "x": xc,
                "seg": seg_h,
                "wi": wi_bf,
                "wh": wh_bf,
                "wz": wz_bf,
                "dw": dw_bf,
                "bi": bi_s,
                "bh": bh_s,
                "bz": bz_s,
                "iota": iota,
            }
        )
    return in_maps


def finish_host(partials, y, W_clf):
    """partials: list of [64, 65] f32 per core -> (loss, logits)."""
    S = np.zeros((N_BAGS, N_PROTO + 1), np.float32)
    for p in partials:
        S = S + np.asarray(p, np.float32)
    counts = S[:, N_PROTO]
    sums = S[:, :N_PROTO] @ np.asarray(W_clf, np.float32).T  # [64, 8]
    logits = sums / np.maximum(counts, 1.0)[:, None]
    m = logits.max(axis=1, keepdims=True)
    lse = m + np.log(np.exp(logits - m).sum(axis=1, keepdims=True))
    logp = logits - lse
    yi = np.asarray(y).astype(np.int64)
    loss = -logp[np.arange(N_BAGS), yi].mean()
    return np.float32(loss), logits.astype(np.float32)


_CACHE = {}


def _get_program():
    if "nc" not in _CACHE:
        _CACHE["nc"] = build_program()
    return _CACHE["nc"]


def kernel(x, y, segment_ids, W_i, b_i, W_h, b_h, W_z, b_z, prototypes, W_clf):
    nc, offs, ng = _get_program()
    in_maps = make_host_inputs(
        x, segment_ids, W_i, b_i, W_h, b_h, W_z, b_z, prototypes
    )
    res = run_bass_kernel_spmd(nc, in_maps, list(range(N_CORES)))
    partials = [res.results[i]["out"] for i in range(N_CORES)]
    return finish_host(partials, y, W_clf)


# revision 10
# speedup vs baseline: 1.5874x; 1.5874x over previous
"""Trainium2 Bass kernel for the vq_codebook bag-classification model.

Math (per reference):
  h1 = lrelu(x @ Wi.T + bi); h2 = lrelu(h1 @ Wh.T + bh); z = lrelu(h2 @ Wz.T + bz)
  c2p = ||z||^2 - 2 z@P.T + ||P||^2 ;  r = 1/(c2p + 0.5)
  c_logits = r @ Wclf.T ; segment-mean over bags ; CE loss.

Device strategy (8 cores, data-parallel over cells):
  - x (fp32, HBM) is loaded with an SWDGE cast-DMA (fp32 -> bf16 inline,
    natural [cells, dims] layout, fully contiguous HBM reads), then each
    [128, 128] block is transposed SBUF->SBUF via the DMA xbar so the
    encoder matmuls get dims-on-partitions bf16 tiles.
  - Encoder runs as bf16 matmuls producing transposed activations
    (feature-on-partition), leaky-relu + bias fused on the Scalar engine.
  - Distances via an augmented matmul: lhsT = [z; z^2; 1; 1] (66 x cells),
    rhs = [-2 P^T; ones; psq_hi; psq_lo] (66 x 64) -> c2p + 0.5 in PSUM.
  - r = reciprocal_approx_fast (DVE). Pooling = matmul with a one-hot
    bag matrix built from segment ids (is_equal against an iota row),
    accumulated in one PSUM bank across the whole kernel -> [64, 65]
    (64 proto-sums per bag + count column).
  - Host: sum the 8 per-core [64,65] partials, apply Wclf, divide by
    counts, log-softmax + NLL. (All-reduce equivalent, trivially small.)
"""

import sys

sys.path.insert(0, "/opt/trn_rl_repo")

import numpy as np
import ml_dtypes

from concourse import bass, bacc, tile, mybir
from concourse.bass_utils import run_bass_kernel_spmd

F32 = mybir.dt.float32
BF16 = mybir.dt.bfloat16

N_CORES = 8
N_CELLS = 200000
D_IN = 1000
H_DIM = 256
Z_DIM = 32
N_PROTO = 64
N_CLASSES = 8
N_BAGS = 64

CELLS_CORE = N_CELLS // N_CORES  # 25000
GROUP = 512  # cells per device group (4 subtiles of 128)

# K-chunking of the 1000-dim input: 7 chunks of 128 + one final chunk
# covering dims 872..999 (its first 24 weight rows are zeroed because dims
# 872..895 are already covered by chunk 6).
K_STARTS = [128 * k for k in range(7)] + [D_IN - 128]


def _group_offsets(cells):
    """Start offsets of 512-cell groups covering [0, cells); the last group
    is shifted back to stay in-bounds (duplicated cells are masked via seg
    ids on the host side)."""
    offs = list(range(0, cells - GROUP + 1, GROUP))
    if offs[-1] + GROUP < cells:
        offs.append(cells - GROUP)
    return offs


def build_program(cells=CELLS_CORE, num_devices=N_CORES, lrelu_native=True):
    offs = _group_offsets(cells)
    ng = len(offs)
    nc = bacc.Bacc(
        "TRN2", target_bir_lowering=False, debug=False, num_devices=num_devices
    )

    x = nc.declare_dram_parameter("x", [cells, D_IN], F32, isOutput=False)
    seg = nc.declare_dram_parameter("seg", [ng, 128, 4], F32, isOutput=False)
    wi = nc.declare_dram_parameter("wi", [8, 128, H_DIM], BF16, isOutput=False)
    wh = nc.declare_dram_parameter("wh", [2, 128, H_DIM], BF16, isOutput=False)
    wz = nc.declare_dram_parameter("wz", [2, 128, Z_DIM], BF16, isOutput=False)
    dw = nc.declare_dram_parameter("dw", [66, N_PROTO], BF16, isOutput=False)
    bi = nc.declare_dram_parameter("bi", [128, 2], F32, isOutput=False)
    bh = nc.declare_dram_parameter("bh", [128, 2], F32, isOutput=False)
    bz = nc.declare_dram_parameter("bz", [Z_DIM, 1], F32, isOutput=False)
    iota = nc.declare_dram_parameter("iota", [128, N_BAGS], F32, isOutput=False)
    out = nc.declare_dram_parameter("out", [N_BAGS, N_PROTO + 1], F32, isOutput=True)

    LRELU = mybir.ActivationFunctionType.Lrelu
    RELU = mybir.ActivationFunctionType.Relu

    ACT_BUFS = 3
    SMALL_BUFS = 3
    with tile.TileContext(nc) as tc:
        with (
            tc.tile_pool(name="const", bufs=1) as cpool,
            tc.tile_pool(name="xt", bufs=3) as xt_pool,
            tc.tile_pool(name="act", bufs=ACT_BUFS) as act_pool,
            tc.tile_pool(name="small", bufs=SMALL_BUFS) as small_pool,
            tc.tile_pool(name="ph1", bufs=1, space="PSUM") as ph1,
            tc.tile_pool(name="ph2", bufs=1, space="PSUM") as ph2,
            tc.tile_pool(name="pz", bufs=1, space="PSUM") as pz,
            tc.tile_pool(name="pd", bufs=1, space="PSUM") as pd,
            tc.tile_pool(name="pacc", bufs=1, space="PSUM") as pacc,
        ):
            # ---- constants / weights into SBUF (one-time) ----
            wi_sb = cpool.tile([128, 8, H_DIM], BF16)
            for k in range(8):
                nc.sync.dma_start(wi_sb[:, k, :], wi[k])
            wh_sb = cpool.tile([128, 2, H_DIM], BF16)
            for k in range(2):
                nc.sync.dma_start(wh_sb[:, k, :], wh[k])
            wz_sb = cpool.tile([128, 2, Z_DIM], BF16)
            for k in range(2):
                nc.sync.dma_start(wz_sb[:, k, :], wz[k])
            dw_sb = cpool.tile([66, N_PROTO], BF16)
            nc.sync.dma_start(dw_sb[:], dw[:])
            bi_sb = cpool.tile([128, 2], F32)
            nc.sync.dma_start(bi_sb[:], bi[:])
            bh_sb = cpool.tile([128, 2], F32)
            nc.sync.dma_start(bh_sb[:], bh[:])
            bz_sb = cpool.tile([Z_DIM, 1], F32)
            nc.sync.dma_start(bz_sb[:], bz[:])
            iota_sb = cpool.tile([128, N_BAGS], F32)
            nc.sync.dma_start(iota_sb[:], iota[:])

            pool_ps = pacc.tile([N_BAGS, N_PROTO + 1], F32)

            def lrelu(dst, src, bias_ap):
                if lrelu_native:
                    nc.scalar.activation(
                        dst, src, LRELU, bias=bias_ap, scale=1.0, alpha=0.01
                    )
                else:
                    # decomposed fallback: lrelu(t) = 0.01*t + 0.99*relu(t)
                    tmp = act_pool.tile(
                        [dst.shape[0], dst.shape[-1]], BF16, tag="lrelu_tmp"
                    )
                    nc.scalar.activation(tmp[:], src, RELU, bias=bias_ap, scale=1.0)
                    lin = act_pool.tile(
                        [dst.shape[0], dst.shape[-1]], BF16, tag="lrelu_lin"
                    )
                    nc.vector.tensor_scalar(
                        lin[:], src, bias_ap, 0.01, mybir.AluOpType.add,
                        mybir.AluOpType.mult,
                    )
                    # dst = 0.99*tmp + lin
                    nc.vector.scalar_tensor_tensor(
                        dst, tmp[:], 0.99, lin[:],
                        op0=mybir.AluOpType.mult, op1=mybir.AluOpType.add,
                    )

            for g, off in enumerate(offs):
                # --- cast-load x naturally (one SWDGE DMA: fp32 -> bf16),
                #     then xbar-transpose each [128,128] block (both HWDGE
                #     rings used in parallel) ---
                nat = xt_pool.tile([128, 4, D_IN], BF16, tag="nat")
                # in_ AP iterates (p, j, d) so that nat[p, j, :] = x[off+128j+p]
                nc.gpsimd.dma_start(
                    nat[:],
                    x[off : off + GROUP].rearrange("(j p) d -> p j d", p=128),
                )
                xt = []
                for k in range(8):
                    t = xt_pool.tile([128, GROUP], BF16, tag=f"xt{k}")
                    s = K_STARTS[k]
                    for cb in range(4):
                        eng = nc.sync if (k * 4 + cb) % 2 == 0 else nc.scalar
                        eng.dma_start(
                            t[:, 128 * cb : 128 * (cb + 1)],
                            nat[:, cb, s : s + 128],
                            transpose=True,
                        )
                    xt.append(t)
                seg_t = small_pool.tile([128, 4], F32, tag="seg")
                nc.sync.dma_start(seg_t[:], seg[g])

                # --- layer 1: h1T[h] [128, 512] = lrelu(Wi @ xT + bi) ---
                h1sb = []
                for h in range(2):
                    ps = ph1.tile([128, GROUP], F32, tag=f"h1_{h}")
                    for k in range(8):
                        nc.tensor.matmul(
                            ps[:],
                            lhsT=wi_sb[:, k, 128 * h : 128 * h + 128],
                            rhs=xt[k][:],
                            start=(k == 0),
                            stop=(k == 7),
                        )
                    sbt = act_pool.tile([128, GROUP], BF16, tag=f"h1sb_{h}")
                    lrelu(sbt[:], ps[:], bi_sb[:, h : h + 1])
                    h1sb.append(sbt)

                # --- layer 2 ---
                h2sb = []
                for h in range(2):
                    ps = ph2.tile([128, GROUP], F32, tag=f"h2_{h}")
                    for k in range(2):
                        nc.tensor.matmul(
                            ps[:],
                            lhsT=wh_sb[:, k, 128 * h : 128 * h + 128],
                            rhs=h1sb[k][:],
                            start=(k == 0),
                            stop=(k == 1),
                        )
                    sbt = act_pool.tile([128, GROUP], BF16, tag=f"h2sb_{h}")
                    lrelu(sbt[:], ps[:], bh_sb[:, h : h + 1])
                    h2sb.append(sbt)

                # --- layer 3 -> dist_in rows 0:32 (z), 32:64 (z^2), 64:66 (1) ---
                zps = pz.tile([Z_DIM, GROUP], F32, tag="z")
                for k in range(2):
                    nc.tensor.matmul(
                        zps[:],
                        lhsT=wz_sb[:, k, :],
                        rhs=h2sb[k][:],
                        start=(k == 0),
                        stop=(k == 1),
                    )
                din = act_pool.tile([66, GROUP], BF16, tag="dist_in")
                if g < ACT_BUFS:
                    # ones rows live in the buffer; written once per slot
                    # (deterministic round-robin slot reuse afterwards)
                    nc.vector.memset(din[64:66, :], 1.0)
                lrelu(din[0:Z_DIM, :], zps[:], bz_sb[:])
                nc.vector.tensor_tensor(
                    din[Z_DIM : 2 * Z_DIM, :],
                    din[0:Z_DIM, :],
                    din[0:Z_DIM, :],
                    mybir.AluOpType.mult,
                )

                # --- distances + 0.5 : c2p [128, 4, 64] per 128-cell subtile ---
                c2p = pd.tile([128, 4, N_PROTO], F32, tag="c2p")
                for j in range(4):
                    nc.tensor.matmul(
                        c2p[:, j, :],
                        lhsT=din[:, 128 * j : 128 * j + 128],
                        rhs=dw_sb[:],
                        start=True,
                        stop=True,
                    )

                # --- r = 1/(c2p+0.5), ones column, one-hot bags ---
                r_sb = small_pool.tile([128, 4, N_PROTO + 1], F32, tag="r")
                if g < SMALL_BUFS:
                    nc.vector.memset(r_sb[:, :, N_PROTO : N_PROTO + 1], 1.0)
                nc.vector.reciprocal_approx_fast(
                    out=r_sb[:, :, 0:N_PROTO], in_=c2p[:]
                )
                oh = small_pool.tile([128, 4, N_BAGS], F32, tag="oh")
                for j in range(4):
                    nc.vector.tensor_scalar(
                        oh[:, j, :],
                        iota_sb[:],
                        seg_t[:, j : j + 1],
                        None,
                        mybir.AluOpType.is_equal,
                    )

                # --- pooled sums += onehot.T @ [r | 1] ---
                for j in range(4):
                    nc.tensor.matmul(
                        pool_ps[:],
                        lhsT=oh[:, j, :],
                        rhs=r_sb[:, j, :],
                        start=(g == 0 and j == 0),
                        stop=(g == ng - 1 and j == 3),
                    )

            out_sb = cpool.tile([N_BAGS, N_PROTO + 1], F32)
            nc.vector.tensor_copy(out_sb[:], pool_ps[:])
            nc.sync.dma_start(out[:], out_sb[:])

    nc.compile()
    return nc, offs, ng


def make_host_inputs(x, segment_ids, W_i, b_i, W_h, b_h, W_z, b_z, prototypes,
                     cells=CELLS_CORE, n_cores=N_CORES):
    """Build the per-core in_maps (and shared weight arrays)."""
    offs = _group_offsets(cells)
    ng = len(offs)

    xv = np.ascontiguousarray(np.asarray(x, dtype=np.float32))

    WiT = np.asarray(W_i, np.float32).T  # [1000, 256]
    wi = np.zeros((8, 128, H_DIM), np.float32)
    for k in range(7):
        wi[k] = WiT[128 * k : 128 * k + 128]
    wi[7] = WiT[D_IN - 128 : D_IN]
    n_overlap = 128 * 7 - (D_IN - 128)  # dims already covered by chunk 6
    wi[7][:n_overlap] = 0.0
    wi_bf = wi.astype(ml_dtypes.bfloat16)

    WhT = np.asarray(W_h, np.float32).T.reshape(2, 128, H_DIM)
    wh_bf = WhT.astype(ml_dtypes.bfloat16)
    WzT = np.asarray(W_z, np.float32).T.reshape(2, 128, Z_DIM)
    wz_bf = WzT.astype(ml_dtypes.bfloat16)

    P = np.asarray(prototypes, np.float32)
    dwf = np.zeros((66, N_PROTO), np.float32)
    dwf[0:32] = -2.0 * P.T
    dwf[32:64] = 1.0
    psq = (P * P).sum(1) + 0.5
    hi = psq.astype(ml_dtypes.bfloat16).astype(np.float32)
    dwf[64] = hi
    dwf[65] = psq - hi
    dw_bf = dwf.astype(ml_dtypes.bfloat16)

    bi_s = np.ascontiguousarray(np.asarray(b_i, np.float32).reshape(2, 128).T)
    bh_s = np.ascontiguousarray(np.asarray(b_h, np.float32).reshape(2, 128).T)
    bz_s = np.asarray(b_z, np.float32).reshape(Z_DIM, 1)
    iota = np.tile(np.arange(N_BAGS, dtype=np.float32), (128, 1))
    iota = np.ascontiguousarray(iota)

    seg_all = np.asarray(segment_ids).astype(np.float32)
    in_maps = []
    for c in range(n_cores):
        xc = xv[c * cells : (c + 1) * cells]
        sc = seg_all[c * cells : (c + 1) * cells]
        segf = np.empty((ng, GROUP), np.float32)
        for g, off in enumerate(offs):
            segf[g] = sc[off : off + GROUP]
        if ng >= 2:
            dup = offs[-2] + GROUP - offs[-1]  # cells of last group already done
            if dup > 0:
                segf[ng - 1, :dup] = -1.0  # matches no bag -> masked out
        seg_h = np.ascontiguousarray(segf.reshape(ng, 4, 128).transpose(0, 2, 1))
        in_maps.append(
            {
                "x": xc,
                "seg": seg_h,
                "wi": wi_bf,
                "wh": wh_bf,
                "wz": wz_bf,
                "dw": dw_bf,
                "bi": bi_s,
                "bh": bh_s,
                "bz": bz_s,
                "iota": iota,
            }
        )
    return in_maps


def finish_host(partials, y, W_clf):
    """partials: list of [64, 65] f32 per core -> (loss, logits)."""
    S = np.zeros((N_BAGS, N_PROTO + 1), np.float32)
    for p in partials:
        S = S + np.asarray(p, np.float32)
    counts = S[:, N_PROTO]
    sums = S[:, :N_PROTO] @ np.asarray(W_clf, np.float32).T  # [64, 8]
    logits = sums / np.maximum(counts, 1.0)[:, None]
    m = logits.max(axis=1, keepdims=True)
    lse = m + np.log(np.exp(logits - m).sum(axis=1, keepdims=True))
    logp = logits - lse
    yi = np.asarray(y).astype(np.int64)
    loss = -logp[np.arange(N_BAGS), yi].mean()
    return np.float32(loss), logits.astype(np.float32)


_CACHE = {}


def _get_program():
    if "nc" not in _CACHE:
        _CACHE["nc"] = build_program()
    return _CACHE["nc"]


def kernel(x, y, segment_ids, W_i, b_i, W_h, b_h, W_z, b_z, prototypes, W_clf):
    nc, offs, ng = _get_program()
    in_maps = make_host_inputs(
        x, segment_ids, W_i, b_i, W_h, b_h, W_z, b_z, prototypes
    )
    res = run_bass_kernel_spmd(nc, in_maps, list(range(N_CORES)))
    partials = [res.results[i]["out"] for i in range(N_CORES)]
    return finish_host(partials, y, W_clf)


# revision 13
# speedup vs baseline: 1.5919x; 1.0029x over previous
"""Trainium2 Bass kernel for the vq_codebook bag-classification model.

Math (per reference):
  h1 = lrelu(x @ Wi.T + bi); h2 = lrelu(h1 @ Wh.T + bh); z = lrelu(h2 @ Wz.T + bz)
  c2p = ||z||^2 - 2 z@P.T + ||P||^2 ;  r = 1/(c2p + 0.5)
  c_logits = r @ Wclf.T ; segment-mean over bags ; CE loss.

Device strategy (8 cores, data-parallel over cells):
  - x (fp32, HBM) is loaded with an SWDGE cast-DMA (fp32 -> bf16 inline,
    natural [cells, dims] layout, fully contiguous HBM reads), then each
    [128, 128] block is transposed SBUF->SBUF via the DMA xbar so the
    encoder matmuls get dims-on-partitions bf16 tiles.
  - Encoder runs as bf16 matmuls producing transposed activations
    (feature-on-partition), leaky-relu + bias fused on the Scalar engine.
  - Distances via an augmented matmul: lhsT = [z; z^2; 1; 1] (66 x cells),
    rhs = [-2 P^T; ones; psq_hi; psq_lo] (66 x 64) -> c2p + 0.5 in PSUM.
  - r = reciprocal_approx_fast (DVE). Pooling = matmul with a one-hot
    bag matrix built from segment ids (is_equal against an iota row),
    accumulated in one PSUM bank across the whole kernel -> [64, 65]
    (64 proto-sums per bag + count column).
  - Host: sum the 8 per-core [64,65] partials, apply Wclf, divide by
    counts, log-softmax + NLL. (All-reduce equivalent, trivially small.)
"""

import sys

sys.path.insert(0, "/opt/trn_rl_repo")

import numpy as np
import ml_dtypes

from concourse import bass, bacc, tile, mybir
from concourse.bass_utils import run_bass_kernel_spmd

F32 = mybir.dt.float32
BF16 = mybir.dt.bfloat16

N_CORES = 8
N_CELLS = 200000
D_IN = 1000
H_DIM = 256
Z_DIM = 32
N_PROTO = 64
N_CLASSES = 8
N_BAGS = 64

CELLS_CORE = N_CELLS // N_CORES  # 25000
GROUP = 512  # cells per device group (4 subtiles of 128)

# K-chunking of the 1000-dim input: 7 chunks of 128 + one final chunk
# covering dims 872..999 (its first 24 weight rows are zeroed because dims
# 872..895 are already covered by chunk 6).
K_STARTS = [128 * k for k in range(7)] + [D_IN - 128]


def _group_offsets(cells):
    """Start offsets of 512-cell groups covering [0, cells); the last group
    is shifted back to stay in-bounds (duplicated cells are masked via seg
    ids on the host side)."""
    offs = list(range(0, cells - GROUP + 1, GROUP))
    if offs[-1] + GROUP < cells:
        offs.append(cells - GROUP)
    return offs


def build_program(cells=CELLS_CORE, num_devices=N_CORES, lrelu_native=True):
    offs = _group_offsets(cells)
    ng = len(offs)
    nc = bacc.Bacc(
        "TRN2", target_bir_lowering=False, debug=False, num_devices=num_devices
    )

    x = nc.declare_dram_parameter("x", [cells, D_IN], F32, isOutput=False)
    seg = nc.declare_dram_parameter("seg", [ng, 128, 4], F32, isOutput=False)
    wi = nc.declare_dram_parameter("wi", [8, 128, H_DIM], BF16, isOutput=False)
    wh = nc.declare_dram_parameter("wh", [2, 128, H_DIM], BF16, isOutput=False)
    wz = nc.declare_dram_parameter("wz", [2, 128, Z_DIM], BF16, isOutput=False)
    dw = nc.declare_dram_parameter("dw", [66, N_PROTO], BF16, isOutput=False)
    bi = nc.declare_dram_parameter("bi", [128, 2], F32, isOutput=False)
    bh = nc.declare_dram_parameter("bh", [128, 2], F32, isOutput=False)
    bz = nc.declare_dram_parameter("bz", [Z_DIM, 1], F32, isOutput=False)
    iota = nc.declare_dram_parameter("iota", [128, N_BAGS], F32, isOutput=False)
    out = nc.declare_dram_parameter("out", [N_BAGS, N_PROTO + 1], F32, isOutput=True)

    LRELU = mybir.ActivationFunctionType.Lrelu
    RELU = mybir.ActivationFunctionType.Relu

    ACT_BUFS = 3
    SMALL_BUFS = 3
    with tile.TileContext(nc) as tc:
        with (
            tc.tile_pool(name="const", bufs=1) as cpool,
            tc.tile_pool(name="xt", bufs=3) as xt_pool,
            tc.tile_pool(name="act", bufs=ACT_BUFS) as act_pool,
            tc.tile_pool(name="small", bufs=SMALL_BUFS) as small_pool,
            tc.tile_pool(name="ph1", bufs=1, space="PSUM") as ph1,
            tc.tile_pool(name="ph2", bufs=1, space="PSUM") as ph2,
            tc.tile_pool(name="pz", bufs=1, space="PSUM") as pz,
            tc.tile_pool(name="pd", bufs=1, space="PSUM") as pd,
            tc.tile_pool(name="pacc", bufs=1, space="PSUM") as pacc,
        ):
            # ---- constants / weights into SBUF (one-time) ----
            wi_sb = cpool.tile([128, 8, H_DIM], BF16)
            for k in range(8):
                nc.sync.dma_start(wi_sb[:, k, :], wi[k])
            wh_sb = cpool.tile([128, 2, H_DIM], BF16)
            for k in range(2):
                nc.sync.dma_start(wh_sb[:, k, :], wh[k])
            wz_sb = cpool.tile([128, 2, Z_DIM], BF16)
            for k in range(2):
                nc.sync.dma_start(wz_sb[:, k, :], wz[k])
            dw_sb = cpool.tile([66, N_PROTO], BF16)
            nc.sync.dma_start(dw_sb[:], dw[:])
            bi_sb = cpool.tile([128, 2], F32)
            nc.sync.dma_start(bi_sb[:], bi[:])
            bh_sb = cpool.tile([128, 2], F32)
            nc.sync.dma_start(bh_sb[:], bh[:])
            bz_sb = cpool.tile([Z_DIM, 1], F32)
            nc.sync.dma_start(bz_sb[:], bz[:])
            iota_sb = cpool.tile([128, N_BAGS], F32)
            nc.sync.dma_start(iota_sb[:], iota[:])

            pool_ps = pacc.tile([N_BAGS, N_PROTO + 1], F32)

            # persistent rotating tiles whose constant regions (ones rows /
            # ones column) are initialized exactly once
            din_tiles = []
            for i in range(ACT_BUFS):
                t = cpool.tile([66, GROUP], BF16, tag=f"din{i}")
                nc.vector.memset(t[64:66, :], 1.0)
                din_tiles.append(t)
            r_tiles = []
            for i in range(SMALL_BUFS):
                t = cpool.tile([128, 4, N_PROTO + 1], F32, tag=f"rt{i}")
                nc.vector.memset(t[:, :, N_PROTO : N_PROTO + 1], 1.0)
                r_tiles.append(t)

            def lrelu(dst, src, bias_ap):
                if lrelu_native:
                    nc.scalar.activation(
                        dst, src, LRELU, bias=bias_ap, scale=1.0, alpha=0.01
                    )
                else:
                    # decomposed fallback: lrelu(t) = 0.01*t + 0.99*relu(t)
                    tmp = act_pool.tile(
                        [dst.shape[0], dst.shape[-1]], BF16, tag="lrelu_tmp"
                    )
                    nc.scalar.activation(tmp[:], src, RELU, bias=bias_ap, scale=1.0)
                    lin = act_pool.tile(
                        [dst.shape[0], dst.shape[-1]], BF16, tag="lrelu_lin"
                    )
                    nc.vector.tensor_scalar(
                        lin[:], src, bias_ap, 0.01, mybir.AluOpType.add,
                        mybir.AluOpType.mult,
                    )
                    # dst = 0.99*tmp + lin
                    nc.vector.scalar_tensor_tensor(
                        dst, tmp[:], 0.99, lin[:],
                        op0=mybir.AluOpType.mult, op1=mybir.AluOpType.add,
                    )

            for g, off in enumerate(offs):
                # --- cast-load x naturally (one SWDGE DMA: fp32 -> bf16),
                #     then xbar-transpose each [128,128] block (both HWDGE
                #     rings used in parallel) ---
                nat = xt_pool.tile([128, 4, D_IN], BF16, tag="nat")
                # in_ AP iterates (p, j, d) so that nat[p, j, :] = x[off+128j+p]
                nc.gpsimd.dma_start(
                    nat[:],
                    x[off : off + GROUP].rearrange("(j p) d -> p j d", p=128),
                )
                xt = []
                for k in range(8):
                    t = xt_pool.tile([128, GROUP], BF16, tag=f"xt{k}")
                    s = K_STARTS[k]
                    for cb in range(4):
                        eng = nc.sync if (k * 4 + cb) % 2 == 0 else nc.scalar
                        eng.dma_start(
                            t[:, 128 * cb : 128 * (cb + 1)],
                            nat[:, cb, s : s + 128],
                            transpose=True,
                        )
                    xt.append(t)
                seg_t = small_pool.tile([128, 4], F32, tag="seg")
                nc.sync.dma_start(seg_t[:], seg[g])

                # --- layer 1: h1T[h] [128, 512] = lrelu(Wi @ xT + bi) ---
                h1sb = []
                for h in range(2):
                    ps = ph1.tile([128, GROUP], F32, tag=f"h1_{h}")
                    for k in range(8):
                        nc.tensor.matmul(
                            ps[:],
                            lhsT=wi_sb[:, k, 128 * h : 128 * h + 128],
                            rhs=xt[k][:],
                            start=(k == 0),
                            stop=(k == 7),
                        )
                    sbt = act_pool.tile([128, GROUP], BF16, tag=f"h1sb_{h}")
                    lrelu(sbt[:], ps[:], bi_sb[:, h : h + 1])
                    h1sb.append(sbt)

                # --- layer 2 ---
                h2sb = []
                for h in range(2):
                    ps = ph2.tile([128, GROUP], F32, tag=f"h2_{h}")
                    for k in range(2):
                        nc.tensor.matmul(
                            ps[:],
                            lhsT=wh_sb[:, k, 128 * h : 128 * h + 128],
                            rhs=h1sb[k][:],
                            start=(k == 0),
                            stop=(k == 1),
                        )
                    sbt = act_pool.tile([128, GROUP], BF16, tag=f"h2sb_{h}")
                    lrelu(sbt[:], ps[:], bh_sb[:, h : h + 1])
                    h2sb.append(sbt)

                # --- layer 3 -> dist_in rows 0:32 (z), 32:64 (z^2), 64:66 (1) ---
                zps = pz.tile([Z_DIM, GROUP], F32, tag="z")
                for k in range(2):
                    nc.tensor.matmul(
                        zps[:],
                        lhsT=wz_sb[:, k, :],
                        rhs=h2sb[k][:],
                        start=(k == 0),
                        stop=(k == 1),
                    )
                din = din_tiles[g % ACT_BUFS]
                lrelu(din[0:Z_DIM, :], zps[:], bz_sb[:])
                nc.vector.tensor_tensor(
                    din[Z_DIM : 2 * Z_DIM, :],
                    din[0:Z_DIM, :],
                    din[0:Z_DIM, :],
                    mybir.AluOpType.mult,
                )

                # --- distances + 0.5 : c2p [128, 4, 64] per 128-cell subtile ---
                c2p = pd.tile([128, 4, N_PROTO], F32, tag="c2p")
                for j in range(4):
                    nc.tensor.matmul(
                        c2p[:, j, :],
                        lhsT=din[:, 128 * j : 128 * j + 128],
                        rhs=dw_sb[:],
                        start=True,
                        stop=True,
                    )

                # --- r = 1/(c2p+0.5), ones column, one-hot bags ---
                r_sb = r_tiles[g % SMALL_BUFS]
                nc.vector.reciprocal_approx_fast(
                    out=r_sb[:, :, 0:N_PROTO], in_=c2p[:]
                )
                oh = small_pool.tile([128, 4, N_BAGS], F32, tag="oh")
                for j in range(4):
                    nc.vector.tensor_scalar(
                        oh[:, j, :],
                        iota_sb[:],
                        seg_t[:, j : j + 1],
                        None,
                        mybir.AluOpType.is_equal,
                    )

                # --- pooled sums += onehot.T @ [r | 1] ---
                for j in range(4):
                    nc.tensor.matmul(
                        pool_ps[:],
                        lhsT=oh[:, j, :],
                        rhs=r_sb[:, j, :],
                        start=(g == 0 and j == 0),
                        stop=(g == ng - 1 and j == 3),
                    )

            out_sb = cpool.tile([N_BAGS, N_PROTO + 1], F32)
            nc.vector.tensor_copy(out_sb[:], pool_ps[:])
            nc.sync.dma_start(out[:], out_sb[:])

    nc.compile()
    return nc, offs, ng


def make_host_inputs(x, segment_ids, W_i, b_i, W_h, b_h, W_z, b_z, prototypes,
                     cells=CELLS_CORE, n_cores=N_CORES):
    """Build the per-core in_maps (and shared weight arrays)."""
    offs = _group_offsets(cells)
    ng = len(offs)

    xv = np.ascontiguousarray(np.asarray(x, dtype=np.float32))

    WiT = np.asarray(W_i, np.float32).T  # [1000, 256]
    wi = np.zeros((8, 128, H_DIM), np.float32)
    for k in range(7):
        wi[k] = WiT[128 * k : 128 * k + 128]
    wi[7] = WiT[D_IN - 128 : D_IN]
    n_overlap = 128 * 7 - (D_IN - 128)  # dims already covered by chunk 6
    wi[7][:n_overlap] = 0.0
    wi_bf = wi.astype(ml_dtypes.bfloat16)

    WhT = np.asarray(W_h, np.float32).T.reshape(2, 128, H_DIM)
    wh_bf = WhT.astype(ml_dtypes.bfloat16)
    WzT = np.asarray(W_z, np.float32).T.reshape(2, 128, Z_DIM)
    wz_bf = WzT.astype(ml_dtypes.bfloat16)

    P = np.asarray(prototypes, np.float32)
    dwf = np.zeros((66, N_PROTO), np.float32)
    dwf[0:32] = -2.0 * P.T
    dwf[32:64] = 1.0
    psq = (P * P).sum(1) + 0.5
    hi = psq.astype(ml_dtypes.bfloat16).astype(np.float32)
    dwf[64] = hi
    dwf[65] = psq - hi
    dw_bf = dwf.astype(ml_dtypes.bfloat16)

    bi_s = np.ascontiguousarray(np.asarray(b_i, np.float32).reshape(2, 128).T)
    bh_s = np.ascontiguousarray(np.asarray(b_h, np.float32).reshape(2, 128).T)
    bz_s = np.asarray(b_z, np.float32).reshape(Z_DIM, 1)
    iota = np.tile(np.arange(N_BAGS, dtype=np.float32), (128, 1))
    iota = np.ascontiguousarray(iota)

    seg_all = np.asarray(segment_ids).astype(np.float32)
    in_maps = []
    for c in range(n_cores):
        xc = xv[c * cells : (c + 1) * cells]
        sc = seg_all[c * cells : (c + 1) * cells]
        segf = np.empty((ng, GROUP), np.float32)
        for g, off in enumerate(offs):
            segf[g] = sc[off : off + GROUP]
        if ng >= 2:
            dup = offs[-2] + GROUP - offs[-1]  # cells of last group already done
            if dup > 0:
                segf[ng - 1, :dup] = -1.0  # matches no bag -> masked out
        seg_h = np.ascontiguousarray(segf.reshape(ng, 4, 128).transpose(0, 2, 1))
        in_maps.append(
            {
                "x": xc,
                "seg": seg_h,
                "wi": wi_bf,
                "wh": wh_bf,
                "wz": wz_bf,
                "dw": dw_bf,
                "bi": bi_s,
                "bh": bh_s,
                "bz": bz_s,
                "iota": iota,
            }
        )
    return in_maps


def finish_host(partials, y, W_clf):
    """partials: list of [64, 65] f32 per core -> (loss, logits)."""
    S = np.zeros((N_BAGS, N_PROTO + 1), np.float32)
    for p in partials:
        S = S + np.asarray(p, np.float32)
    counts = S[:, N_PROTO]
    sums = S[:, :N_PROTO] @ np.asarray(W_clf, np.float32).T  # [64, 8]
    logits = sums / np.maximum(counts, 1.0)[:, None]
    m = logits.max(axis=1, keepdims=True)
    lse = m + np.log(np.exp(logits - m).sum(axis=1, keepdims=True))
    logp = logits - lse
    yi = np.asarray(y).astype(np.int64)
    loss = -logp[np.arange(N_BAGS), yi].mean()
    return np.float32(loss), logits.astype(np.float32)


_CACHE = {}


def _get_program():
    if "nc" not in _CACHE:
        _CACHE["nc"] = build_program()
    return _CACHE["nc"]


def kernel(x, y, segment_ids, W_i, b_i, W_h, b_h, W_z, b_z, prototypes, W_clf):
    nc, offs, ng = _get_program()
    in_maps = make_host_inputs(
        x, segment_ids, W_i, b_i, W_h, b_h, W_z, b_z, prototypes
    )
    res = run_bass_kernel_spmd(nc, in_maps, list(range(N_CORES)))
    partials = [res.results[i]["out"] for i in range(N_CORES)]
    return finish_host(partials, y, W_clf)
